# revision 1
# baseline (speedup 1.0000x reference)
"""Raw-Bass Trainium2 kernel for windowed head-axis attention module.

Computation (per batch b, one NeuronCore each, 8 cores):
  qkv = x @ qkv_w.T + qkv_b                  (s, 3D)
  q,k,v -> per position p: scores[h,g] = q_p[h]·k_p[g]/sqrt(DH) over DH=80
  attn = softmax_g(scores); out2_p[h,:] = sum_g attn[h,g] v_p[g,:]
  X_proj[(n,h), (w,d)] = out2;  out = X_proj @ proj_w.T + proj_b

Implementation: all matmuls bf16 (fp32 PSUM accumulate). Attention packs 8
positions into one 128x128 PE matmul over the (pos,head) axes; cross-position
pairs are removed with a multiplicative block mask after exp (no max
subtraction needed: |scores| <= ~4). The qkv weights are column-permuted on
the host (feat' = h*240 + j*80 + d) so one 3-dim DMA re-partitions a group's
q|k|v into attention layout. Projection interleaves after every 8 windows via
on-chip PE transposes of the accumulated 128-row block, with proj weights
streamed in 4-k-tile chunks.

Raw bass (no Tile): this toolchain's walrus allows at most ONE embedded sem
wait per instruction, so all synchronization is explicit wait_ge instructions
+ counting semaphores with python-side tick bookkeeping. Attention runs in
per-window stages (each engine sweeps all 8 groups per stage) so cross-engine
handoff latency amortizes over a window instead of per group, and PE runs
qkv[st+1] before attention[st] to hide the scatter-DMA latency.
"""

import numpy as np
import ml_dtypes

import concourse.bass as bass
import concourse.mybir as mybir
from concourse.bass_utils import run_bass_kernel_spmd

BF16 = mybir.dt.bfloat16
F32 = mybir.dt.float32

D, H, DH, W = 1280, 16, 80, 64
S, B = 4096, 8
NWIN = S // W            # 64 windows
D3 = 3 * D               # 3840
PROJ_K = W * DH          # 5120
OUT_ROWS = NWIN * H      # 1024
SCALE = 1.0 / float(np.sqrt(np.float32(DH)))

N_CORES = 8
NST = S // 128           # 32 s-tiles
KT = D // 128            # 10
PKT = PROJ_K // 128      # 40
NSL = [(i * 512, min(512, D3 - i * 512)) for i in range(8)]
NSLO = [(i * 512, min(512, D - i * 512)) for i in range(3)]
NMB = NST // 4           # 8 m-blocks (128 output rows each)


def build_nc(with_qkv_bias: bool):
    nc = bass.Bass("TRN2", debug=False, num_devices=N_CORES)

    xT_d = nc.dram_tensor("xT", [D, S], BF16, kind="ExternalInput")
    qkv_wT_d = nc.dram_tensor("qkv_wT", [D, D3], BF16, kind="ExternalInput")
    proj_wT_d = nc.dram_tensor("proj_wT", [PROJ_K, D], BF16, kind="ExternalInput")
    mask_d = nc.dram_tensor("mask", [128, 128], BF16, kind="ExternalInput")
    if with_qkv_bias:
        qkvb_d = nc.dram_tensor("qkv_b", [1, D3], BF16, kind="ExternalInput")
    out_d = nc.dram_tensor("out", [OUT_ROWS, D], F32, kind="ExternalOutput")

    xT_v = xT_d.ap().rearrange("(k p) s -> p k s", p=128)
    qkv_wT_v = qkv_wT_d.ap().rearrange("(k p) n -> p k n", p=128)
    pw_chunk_v = proj_wT_d.ap().rearrange("(c k4 p) n -> c p k4 n", k4=4, p=128)

    # ---------------- tick prepass (must mirror emission order) ----------
    pe_t, dve_t, act_t = {}, {}, {}
    counters = {"pe": 0, "dve": 0, "act": 0}

    PE_STAGE_KEY = {1: "tp", 2: "sc", 3: "emtp", 4: "att"}
    # interleave plan for a PE block: qkv slices of st mixed with window
    # stages of st-1 (windows a=2(st-1), b=2(st-1)+1)
    PE_SEQ = [("q", 0), ("q", 1), ("a", 1), ("q", 2), ("a", 2), ("q", 3),
              ("a", 3), ("q", 4), ("a", 4), ("q", 5), ("b", 1), ("q", 6),
              ("b", 2), ("q", 7), ("b", 3), ("b", 4)]

    def pe_stage_ticks(n, stg):
        key = PE_STAGE_KEY[stg]
        for g in range(8):
            counters["pe"] += 1
            pe_t[(key, n, g)] = counters["pe"]

    def pe_mblock_ticks(m):
        for key in ("xpst", "pjk"):
            for k in range(PKT):
                counters["pe"] += 1
                pe_t[(key, m, k)] = counters["pe"]

    for st in range(NST):
        if st == 0:
            for i in range(8):
                counters["pe"] += 1
                pe_t[("qkv", 0, i)] = counters["pe"]
            continue
        a, b = (st - 1) * 2, (st - 1) * 2 + 1
        for kind, idx in PE_SEQ:
            if kind == "q":
                counters["pe"] += 1
                pe_t[("qkv", st, idx)] = counters["pe"]
            elif kind == "a":
                pe_stage_ticks(a, idx)
            else:
                pe_stage_ticks(b, idx)
        if (st - 1) % 4 == 3:
            pe_mblock_ticks((st - 1) // 4)
    for stg in (1, 2, 3, 4):
        pe_stage_ticks((NST - 1) * 2, stg)
    for stg in (1, 2, 3, 4):
        pe_stage_ticks((NST - 1) * 2 + 1, stg)
    pe_mblock_ticks(NMB - 1)

    # DVE block plan: cp slices of st mixed with window stages of st-1
    DVE_SEQ = [("q", 0), ("a", 0), ("q", 1), ("a", 1), ("a", 2), ("q", 2),
               ("a", 3), ("q", 3), ("a", 4), ("q", 4), ("b", 0), ("q", 5),
               ("b", 1), ("b", 2), ("q", 6), ("b", 3), ("q", 7), ("b", 4)]

    def dve_stage_ticks(n, stg):
        if stg == 0:
            for gp in range(4):
                counters["dve"] += 1
                dve_t[("qkb", n, gp)] = counters["dve"]
        elif stg == 1:
            for g in range(8):
                counters["dve"] += 1
                dve_t[("em", n, g)] = counters["dve"]
        elif stg == 2:
            counters["dve"] += 1
            dve_t[("rc", n)] = counters["dve"]
        elif stg == 3:
            for gp in range(4):
                counters["dve"] += 1
                dve_t[("emtc", n, gp)] = counters["dve"]
        else:
            for g in range(8):
                counters["dve"] += 1
                dve_t[("ts", n, g)] = counters["dve"]

    def dve_mblock_ticks(m):
        for k in range(PKT):
            counters["dve"] += 1
            dve_t[("xptc", m, k)] = counters["dve"]
        for i in range(3):
            counters["dve"] += 1
            dve_t[("oc", m, i)] = counters["dve"]

    for st in range(NST):
        if st == 0:
            for i in range(8):
                counters["dve"] += 1
                dve_t[("cp", 0, i)] = counters["dve"]
            continue
        a, b = (st - 1) * 2, (st - 1) * 2 + 1
        for kind, idx in DVE_SEQ:
            if kind == "q":
                counters["dve"] += 1
                dve_t[("cp", st, idx)] = counters["dve"]
            elif kind == "a":
                dve_stage_ticks(a, idx)
            else:
                dve_stage_ticks(b, idx)
        if (st - 1) % 4 == 3:
            dve_mblock_ticks((st - 1) // 4)
    for stg in (0, 1, 2, 3, 4):
        dve_stage_ticks((NST - 1) * 2, stg)
    for stg in (0, 1, 2, 3, 4):
        dve_stage_ticks((NST - 1) * 2 + 1, stg)
    dve_mblock_ticks(NMB - 1)

    for n in range(NWIN):
        for gp in range(4):
            counters["act"] += 1
            act_t[("exp", n, gp)] = counters["act"]

    from contextlib import ExitStack
    ctx = ExitStack()
    with ctx:
        E = ctx.enter_context
        # ---- SBUF (static) ----
        wq_sb = E(nc.sbuf_tensor([128, KT, D3], BF16))
        mask_sb = E(nc.sbuf_tensor([128, 128], BF16))
        ident = E(nc.sbuf_tensor([128, 128], BF16))
        xt_sb = E(nc.sbuf_tensor([128, 2, KT, 128], BF16))
        qkv_sb = E(nc.sbuf_tensor([128, 2, D3], BF16))
        wide = E(nc.sbuf_tensor([128, 2, 8 * 240], BF16))
        qkb = E(nc.sbuf_tensor([128, 8, 256], BF16))
        e_sb = E(nc.sbuf_tensor([128, 8, 128], BF16))
        em_sb = E(nc.sbuf_tensor([128, 8, 128], BF16))
        r_sb = E(nc.sbuf_tensor([128, 8, 1], F32))
        rinv_sb = E(nc.sbuf_tensor([128, 8, 1], F32))
        emt_sb = E(nc.sbuf_tensor([128, 8, 128], BF16))
        wbuf = E(nc.sbuf_tensor([128, 2, 640], BF16))
        xps = E(nc.sbuf_tensor([128, PROJ_K], BF16))
        xpt = E(nc.sbuf_tensor([128, PKT, 128], BF16))
        pw_sb = E(nc.sbuf_tensor([128, 5, 4, D], BF16))
        o_sb = E(nc.sbuf_tensor([128, 3, 512], F32))
        qkvb_sb = E(nc.sbuf_tensor([1, D3 if with_qkv_bias else 1], BF16))
        ones_sb = E(nc.sbuf_tensor([1, 128], BF16))
        # ---- PSUM: 8 banks ----
        qkv_ps = E(nc.psum_tensor([128, 2, 512], F32))
        tp_ps = E(nc.psum_tensor([128, 2, 1024], BF16))
        sc_ps = E(nc.psum_tensor([128, 2, 512], F32))
        o_ps = E(nc.psum_tensor([128, 2, 512], F32))
        # ---- semaphores ----
        sW = E(nc.semaphore())
        sPOOL = E(nc.semaphore())
        sPE = E(nc.semaphore())
        sDVE = E(nc.semaphore())
        sACT = E(nc.semaphore())
        sDX0 = E(nc.semaphore()); sDX1 = E(nc.semaphore())
        sDS0 = E(nc.semaphore()); sDS1 = E(nc.semaphore())
        sDXP0 = E(nc.semaphore()); sDXP1 = E(nc.semaphore())
        sDPW0 = E(nc.semaphore()); sDPW1 = E(nc.semaphore())
        sDPW2 = E(nc.semaphore()); sDPW3 = E(nc.semaphore())
        sDPW4 = E(nc.semaphore())
        sDO = E(nc.semaphore())
        block = E(nc.Block())
        sDX = [sDX0, sDX1]
        sDS = [sDS0, sDS1]
        sDXP = [sDXP0, sDXP1]
        sDPW = [sDPW0, sDPW1, sDPW2, sDPW3, sDPW4]

        def waiter(eng):
            seen = {}
            def w(sem, val):
                if val is None or val <= 0:
                    return
                if seen.get(id(sem), -1) >= val:
                    return
                seen[id(sem)] = val
                eng.wait_ge(sem, val)
            return w

        def ds_target(n):        # all 8 scatters of window n done
            return 16 * 8 * (n // 2 + 1)

        def dxp_target(n):       # all xps writes of same-parity windows <= n
            return 16 * 8 * (n // 2 + 1)

        n_w_dmas = 3 if with_qkv_bias else 2

        # ================= SP: HWDGE DMA issue =================
        def sp_prog(sp):
            w = waiter(sp)
            sp.dma_start(wq_sb[:, :, :], qkv_wT_v).then_inc(sW, 16)
            sp.dma_start(mask_sb[:, :], mask_d.ap()).then_inc(sW, 16)
            if with_qkv_bias:
                sp.dma_start(qkvb_sb[:, :], qkvb_d.ap()).then_inc(sW, 16)
            sp.dma_start(xt_sb[:, 0, :, :], xT_v[:, :, 0:128]).then_inc(sDX[0], 16)
            sp.dma_start(xt_sb[:, 1, :, :], xT_v[:, :, 128:256]).then_inc(sDX[1], 16)
            for st in range(NST):
                for win in range(2):
                    n = st * 2 + win
                    w(sDVE, dve_t[("cp", st, 7)])
                    if n >= 2:
                        w(sPE, pe_t[("att", n - 2, 7)])
                    for g in range(8):
                        g0 = win * 64 + g * 8
                        c0 = g * 240
                        src = qkv_sb[g0 : g0 + 8, st % 2, :].rearrange(
                            "w (h c) -> w h c", h=H
                        )
                        sp.dma_start(
                            wide[:, n % 2, c0 : c0 + 240], src
                        ).then_inc(sDS[n % 2], 16)
                    if win == 0 and st + 2 < NST:
                        w(sPE, pe_t[("qkv", st, 7)])
                        sp.dma_start(
                            xt_sb[:, st % 2, :, :],
                            xT_v[:, :, (st + 2) * 128 : (st + 3) * 128],
                        ).then_inc(sDX[st % 2], 16)
                if st % 4 == 2:
                    m = st // 4
                    for c in range(5):
                        if m >= 1:
                            w(sPE, pe_t[("pjk", m - 1, 4 * (c + 5) + 3)])
                        sp.dma_start(
                            pw_sb[:, c % 5, :, :], pw_chunk_v[c]
                        ).then_inc(sDPW[c % 5], 16)
                if st % 4 == 3:
                    m = st // 4
                    for c in range(5, PKT // 4):
                        w(sPE, pe_t[("pjk", m, 4 * (c - 5) + 3)])
                        sp.dma_start(
                            pw_sb[:, c % 5, :, :], pw_chunk_v[c]
                        ).then_inc(sDPW[c % 5], 16)
                    for i, (no, nw_) in enumerate(NSLO):
                        w(sDVE, dve_t[("oc", m, i)])
                        sp.dma_start(
                            out_d.ap()[m * 128 : (m + 1) * 128, no : no + nw_],
                            o_sb[:, i, :nw_],
                        ).then_inc(sDO, 16)
            w(sDO, 16 * 3 * NMB)

        # ================= PE =================
        def pe_prog(tn):
            w = waiter(tn)

            def pe_qkv_slice(st, i):
                no, nw_ = NSL[i]
                w(sDX[st % 2], 16 * (st // 2 + 1))
                if i >= 2:
                    w(sDVE, dve_t[("cp", st, i - 2)])
                elif st >= 1:
                    w(sDVE, dve_t[("cp", st - 1, i + 6)])
                if i == 0 and st >= 5 and st % 4 == 1:
                    # qkv_ps[0] was proj accumulator of m-block (st-5)//4
                    w(sDVE, dve_t[("oc", (st - 5) // 4, 0)])
                ps = qkv_ps[:, i % 2, :nw_]
                for k in range(KT):
                    mm = tn.matmul(
                        ps,
                        xt_sb[:, st % 2, k, :],
                        wq_sb[:, k, no : no + nw_],
                        start=(k == 0),
                        stop=(k == KT - 1) and not with_qkv_bias,
                    )
                if with_qkv_bias:
                    mm = tn.matmul(
                        ps, ones_sb[:, :], qkvb_sb[:, no : no + nw_],
                        start=False, stop=True,
                    )
                mm.then_inc(sPE, 1)

            def pe_stage(n, stg):
                if stg == 1:
                    w(sDS[n % 2], ds_target(n))
                    if n >= 1:
                        w(sDVE, dve_t[("emtc", n - 1, 3)])
                    for g in range(8):
                        c0 = g * 240
                        gs = g % 2
                        if g >= 2:
                            w(sDVE, dve_t[("qkb", n, (g - 2) // 2)])
                        tn.transpose(
                            tp_ps[0:80, gs, 0:128],
                            wide[:, n % 2, c0 : c0 + 80],
                            ident[:, :],
                        )
                        tn.transpose(
                            tp_ps[0:80, gs, 128:256],
                            wide[:, n % 2, c0 + 80 : c0 + 160],
                            ident[:, :],
                        ).then_inc(sPE, 1)
                elif stg == 2:
                    for g in range(8):
                        gs = g % 2
                        w(sDVE, dve_t[("qkb", n, g // 2)])
                        if g >= 2:
                            w(sACT, act_t[("exp", n, (g - 2) // 2)])
                        elif n >= 1:
                            w(sACT, act_t[("exp", n - 1, 3)])
                        if gs == 0 and n % 8 == 0 and n >= 8:
                            w(sDVE, dve_t[("oc", n // 8 - 1, 1)])
                        tn.matmul(
                            sc_ps[:, gs, 0:128],
                            qkb[0:80, g, 0:128],
                            qkb[0:80, g, 128:256],
                            start=True,
                            stop=True,
                        ).then_inc(sPE, 1)
                elif stg == 3:
                    for g in range(8):
                        gs = g % 2
                        w(sDVE, dve_t[("em", n, g)])
                        if g >= 2:
                            w(sDVE, dve_t[("emtc", n, (g - 2) // 2)])
                        tn.transpose(
                            tp_ps[:, gs, 256:384], em_sb[:, g, :], ident[:, :]
                        ).then_inc(sPE, 1)
                else:
                    for g in range(8):
                        gs = g % 2
                        c0 = g * 240
                        w(sDVE, dve_t[("emtc", n, g // 2)])
                        if g >= 2:
                            w(sDVE, dve_t[("ts", n, g - 2)])
                        elif n >= 1:
                            w(sDVE, dve_t[("ts", n - 1, g + 6)])
                        if gs == 0 and n % 8 == 0 and n >= 8:
                            w(sDVE, dve_t[("oc", n // 8 - 1, 2)])
                        tn.matmul(
                            o_ps[:, gs, 0:80],
                            emt_sb[:, g, :],
                            wide[:, n % 2, c0 + 160 : c0 + 240],
                            start=True,
                            stop=True,
                        ).then_inc(sPE, 1)

            def pe_mblock(m):
                nlast = 8 * m + 7
                w(sDXP[0], 16 * 8 * 4 * (m + 1))
                w(sDXP[1], 16 * 8 * 4 * (m + 1))
                w(sDVE, dve_t[("emtc", nlast, 3)])
                for k in range(PKT):
                    ks = k % 2
                    if k >= 2:
                        w(sDVE, dve_t[("xptc", m, k - 2)])
                    tn.transpose(
                        tp_ps[:, ks, 0:128],
                        xps[:, k * 128 : (k + 1) * 128],
                        ident[:, :],
                    ).then_inc(sPE, 1)
                # proj: 3 psum accumulators (qkv_ps[0], sc_ps[0], o_ps[0])
                stq = min(4 * m + 4, NST - 1)   # latest qkv before this proj
                w(sDVE, dve_t[("cp", stq, 6)])
                w(sACT, act_t[("exp", nlast, 3)])
                w(sDVE, dve_t[("ts", nlast, 6)])
                accs = [qkv_ps[:, 0, :], sc_ps[:, 0, :], o_ps[:, 0, :]]
                for k in range(PKT):
                    c = k // 4
                    w(sDPW[c % 5], 16 * (2 * m + c // 5 + 1))
                    w(sDVE, dve_t[("xptc", m, k)])
                    for i, (no, nw_) in enumerate(NSLO):
                        mm = tn.matmul(
                            accs[i][:, :nw_],
                            xpt[:, k, :],
                            pw_sb[:, c % 5, k % 4, no : no + nw_],
                            start=(k == 0),
                            stop=(k == PKT - 1),
                        )
                    mm.then_inc(sPE, 1)

            w(sW, 16 * n_w_dmas)
            w(sPOOL, 3 if with_qkv_bias else 2)
            for st in range(NST):
                if st == 0:
                    for i in range(8):
                        pe_qkv_slice(0, i)
                    continue
                a, b = (st - 1) * 2, (st - 1) * 2 + 1
                for kind, idx in PE_SEQ:
                    if kind == "q":
                        pe_qkv_slice(st, idx)
                    elif kind == "a":
                        pe_stage(a, idx)
                    else:
                        pe_stage(b, idx)
                if (st - 1) % 4 == 3:
                    pe_mblock((st - 1) // 4)
            for stg in (1, 2, 3, 4):
                pe_stage((NST - 1) * 2, stg)
            for stg in (1, 2, 3, 4):
                pe_stage((NST - 1) * 2 + 1, stg)
            pe_mblock(NMB - 1)

        # ================= DVE =================
        def dve_prog(dv):
            w = waiter(dv)

            def dve_cp_slice(st, i):
                no, nw_ = NSL[i]
                w(sPE, pe_t[("qkv", st, i)])
                if i == 0 and st >= 2:
                    w(sDS[0], ds_target(2 * (st - 2)))
                    w(sDS[1], ds_target(2 * (st - 2) + 1))
                dv.tensor_copy(
                    qkv_sb[:, st % 2, no : no + nw_], qkv_ps[:, i % 2, :nw_]
                ).then_inc(sDVE, 1)

            def dve_stage(n, stg):
                if stg == 0:
                    for gp in range(4):
                        w(sPE, pe_t[("tp", n, 2 * gp + 1)])
                        if n >= 1:
                            w(sPE, pe_t[("sc", n - 1, 2 * gp + 1)])
                        dv.tensor_copy(
                            qkb[0:80, 2 * gp : 2 * gp + 2, :],
                            tp_ps[0:80, :, 0:256],
                        ).then_inc(sDVE, 1)
                elif stg == 1:
                    for g in range(8):
                        w(sACT, act_t[("exp", n, g // 2)])
                        if n >= 1:
                            w(sPE, pe_t[("emtp", n - 1, g)])
                        dv.scalar_tensor_tensor(
                            out=em_sb[:, g, :],
                            in0=e_sb[:, g, :],
                            scalar=1.0,
                            in1=mask_sb[:, :],
                            op0=mybir.AluOpType.mult,
                            op1=mybir.AluOpType.mult,
                            accum_out=r_sb[:, g, :],
                        ).then_inc(sDVE, 1)
                elif stg == 2:
                    w(sDVE, dve_t[("em", n, 7)])
                    dv.reciprocal(
                        rinv_sb[:, 0:8, :], r_sb[:, 0:8, :]
                    ).then_inc(sDVE, 1)
                elif stg == 3:
                    for gp in range(4):
                        w(sPE, pe_t[("emtp", n, 2 * gp + 1)])
                        if n >= 1:
                            w(sPE, pe_t[("att", n - 1, 2 * gp + 1)])
                        dv.tensor_copy(
                            emt_sb[:, 2 * gp : 2 * gp + 2, :], tp_ps[:, :, 256:384]
                        ).then_inc(sDVE, 1)
                else:
                    for g in range(8):
                        w(sPE, pe_t[("att", n, g)])
                        w(sDVE, dve_t[("rc", n)])
                        if n >= 2 and g == 0:
                            w(sDXP[n % 2], dxp_target(n - 2))
                        dv.tensor_scalar_mul(
                            wbuf[:, n % 2, g * DH : (g + 1) * DH],
                            o_ps[:, g % 2, 0:80],
                            rinv_sb[:, g, :],
                        ).then_inc(sDVE, 1)

            def dve_mblock(m):
                for k in range(PKT):
                    w(sPE, pe_t[("xpst", m, k)])
                    if m >= 1:
                        w(sPE, pe_t[("pjk", m - 1, k)])
                    dv.tensor_copy(
                        xpt[:, k, :], tp_ps[:, k % 2, 0:128]
                    ).then_inc(sDVE, 1)
                for i, (no, nw_) in enumerate(NSLO):
                    w(sPE, pe_t[("pjk", m, PKT - 1)])
                    if m >= 1:
                        w(sDO, 16 * 3 * m)
                    srcs = [qkv_ps[:, 0, :nw_], sc_ps[:, 0, :nw_], o_ps[:, 0, :nw_]]
                    dv.tensor_copy(o_sb[:, i, :nw_], srcs[i]).then_inc(sDVE, 1)

            w(sW, 16 * n_w_dmas)
            for st in range(NST):
                if st == 0:
                    for i in range(8):
                        dve_cp_slice(0, i)
                    continue
                a, b = (st - 1) * 2, (st - 1) * 2 + 1
                for kind, idx in DVE_SEQ:
                    if kind == "q":
                        dve_cp_slice(st, idx)
                    elif kind == "a":
                        dve_stage(a, idx)
                    else:
                        dve_stage(b, idx)
                if (st - 1) % 4 == 3:
                    dve_mblock((st - 1) // 4)
            for stg in (0, 1, 2, 3, 4):
                dve_stage((NST - 1) * 2, stg)
            for stg in (0, 1, 2, 3, 4):
                dve_stage((NST - 1) * 2 + 1, stg)
            dve_mblock(NMB - 1)

        # ================= ACT =================
        def act_prog(ac):
            w = waiter(ac)
            for n in range(NWIN):
                for gp in range(4):
                    w(sPE, pe_t[("sc", n, 2 * gp + 1)])
                    if n >= 1:
                        w(sDVE, dve_t[("em", n - 1, 2 * gp + 1)])
                    ac.activation(
                        e_sb[:, 2 * gp : 2 * gp + 2, :],
                        sc_ps[:, :, 0:128],
                        mybir.ActivationFunctionType.Exp,
                        scale=SCALE,
                    ).then_inc(sACT, 1)

        # ================= POOL: identity + xps assembly =================
        def pool_prog(pl):
            w = waiter(pl)
            pl.memset(ident[:, :], 0.0).then_inc(sPOOL, 1)
            pl.wait_ge(sPOOL, 1)
            pl.affine_select(
                out=ident[:, :],
                in_=ident[:, :],
                compare_op=mybir.AluOpType.not_equal,
                fill=1.0,
                base=0,
                pattern=[[-1, 128]],
                channel_multiplier=1,
            ).then_inc(sPOOL, 1)
            if with_qkv_bias:
                pl.memset(ones_sb[:, :], 1.0).then_inc(sPOOL, 1)
            for n in range(NWIN):
                w(sDVE, dve_t[("ts", n, 7)])
                if n % 8 == 0 and n // 8 >= 1:
                    w(sPE, pe_t[("xpst", n // 8 - 1, PKT - 1)])
                band = (n % 8) * 16
                dstv = xps[band : band + 16, :].rearrange(
                    "p (g w d) -> p w g d", g=8, w=8, d=DH
                )
                for ww in range(8):
                    pl.dma_start(
                        dstv[:, ww],
                        wbuf[ww * 16 : (ww + 1) * 16, n % 2, :].rearrange(
                            "p (g d) -> p g d", g=8
                        ),
                    ).then_inc(sDXP[n % 2], 16)

        block.gpsimd(pool_prog)
        block.sync(sp_prog)
        block.tensor(pe_prog)
        block.vector(dve_prog)
        block.scalar(act_prog)

    return nc


_CACHE = {}


def _get_nc(with_qkv_bias: bool):
    if with_qkv_bias not in _CACHE:
        _CACHE[with_qkv_bias] = build_nc(with_qkv_bias)
    return _CACHE[with_qkv_bias]


def _perm():
    h_, j_, d_ = np.meshgrid(np.arange(H), np.arange(3), np.arange(DH),
                             indexing="ij")
    return (j_ * D + h_ * DH + d_).reshape(-1)


def kernel(x, cu_seqlens, qkv_w, qkv_b, proj_w, proj_b):
    x = np.asarray(x, dtype=np.float32)
    qkv_w = np.asarray(qkv_w, dtype=np.float32)
    qkv_b = np.asarray(qkv_b, dtype=np.float32)
    proj_w = np.asarray(proj_w, dtype=np.float32)
    proj_b = np.asarray(proj_b, dtype=np.float32)

    bf = ml_dtypes.bfloat16
    xT = np.ascontiguousarray(x.transpose(1, 2, 0)).astype(bf)      # (B, D, S)
    PERM = _perm()
    qkv_wT = np.ascontiguousarray(qkv_w.T[:, PERM]).astype(bf)      # (D, 3D)
    proj_wT = np.ascontiguousarray(proj_w.T).astype(bf)             # (5120, D)
    blk = np.arange(128) // 16
    mask = (blk[:, None] == blk[None, :]).astype(bf)

    with_bias = bool(np.any(qkv_b != 0.0))
    nc = _get_nc(with_bias)

    in_maps = []
    for c in range(N_CORES):
        m = {
            "xT": np.ascontiguousarray(xT[c]),
            "qkv_wT": qkv_wT,
            "proj_wT": proj_wT,
            "mask": mask,
        }
        if with_bias:
            m["qkv_b"] = qkv_b[PERM].astype(bf).reshape(1, D3)
        in_maps.append(m)

    res = run_bass_kernel_spmd(nc, in_maps, core_ids=list(range(N_CORES)))
    out = np.empty((OUT_ROWS, B, D), dtype=np.float32)
    for c in range(N_CORES):
        out[:, c, :] = res.results[c]["out"]
    out += proj_b[None, None, :]
    return out

